# revision 25
# baseline (speedup 1.0000x reference)
"""Trainium2 Bass kernel for the BinaryMechanismSSM problem.

Full inputs in, full outputs out. Internally: batch (128) sharded 8 ways
(16 rows/core). Per core:
  Phase 1 (paced into phase-2 idle gaps): projections
    bx_m = 512*(x @ Wm^T + bm)  (fp16 staging planes, one per (m, j-block)),
    g = sigmoid(x @ G^T + gb)   (fp16 planes per j-block).
    fp16 matmuls with N=1024 token tiles; psum drains (bias add / sigmoid)
    run on ACT (Identity/Sigmoid + per-partition bias) at demoted priority
    so they fill the ACT idle window between recurrence tanhs.
  Phase 2: T sequential steps. State st[p, j*16+b] = s[b, 128j+p] held in
    fp16 [128, 64]. Per step: one fp16 identity matmul injects the staged
    512*bx planes into PSUM [128, (m,j,b)], then 32 fp8(e4m3, x512) weight
    matmuls accumulate A_m @ s in k-major rounds (E bank j23 first); two
    ACT tanh ops (scale=1/512) over psum halves; per-half DVE blend (fp16)
    with precomputed gate coefficient planes; m2=(1-g)s on Pool; fp16
    state written into an 8-step staging tile DMA'd to DRAM every 8 steps.
    Host re-layouts to [B, T+1, S] f32.
"""
import numpy as np
import ml_dtypes

B_FULL = 128
T_FULL = 1024
I_DIM = 256
S_DIM = 512
N_CORES = 8
B_LOC = B_FULL // N_CORES  # 16

_cache = {}


def _build(alpha: float, z: int, T: int):
    import concourse.bass as bass
    from concourse import bacc
    import concourse.mybir as mybir
    from concourse.tile import TileContext

    dt = mybir.dt
    AF = mybir.ActivationFunctionType
    ALU = mybir.AluOpType

    TOK = T * B_LOC          # tokens per core
    TT = 512                 # phase-1 token tile (psum bank limit: N<=512)
    NTT = max(1, TOK // TT)  # phase-1 token tiles
    NG = T // 16             # phase-2 step groups
    GPT = TT // 256          # groups covered per token tile (2)
    NREC = 2 if z != 0 else 1
    NMAT = NREC + 1
    W = NREC * 64            # psum width per step
    SC = 512.0               # fp8/bx prescale

    a0 = float(1.0 - alpha) if z != 0 else 1.0
    a1 = float(alpha)
    # blend fold: f = a0*f0 + a1*f1; v = f0 + r*f1, u = gbase*g*v
    if z != 0 and a0 >= 1e-6:
        gbase, rfold = a0, a1 / a0
    elif z != 0:
        gbase, rfold = a1, None  # alpha == 1: f = f1 only
    else:
        gbase, rfold = 1.0, None

    nc = bacc.Bacc("TRN2", target_bir_lowering=False, debug=False,
                   num_devices=N_CORES)

    xT_d = nc.declare_dram_parameter("xT", [2, 128, TOK], dt.float16, isOutput=False)
    pw_d = nc.declare_dram_parameter("pw", [128, NMAT * 2 * 4 * 128], dt.float16, isOutput=False)
    bias_d = nc.declare_dram_parameter("bias", [128, 4 * NMAT], dt.float32, isOutput=False)
    aw_d = nc.declare_dram_parameter("aw", [128, NREC * 16 * 128], dt.float8e4, isOutput=False)
    s0_d = nc.declare_dram_parameter("s0T", [128, 64], dt.float16, isOutput=False)
    iden_d = nc.declare_dram_parameter("iden", [128, 128], dt.float16, isOutput=False)
    stg_d = nc.declare_dram_parameter("stg", [128, T * 64], dt.float16, isOutput=True)

    with TileContext(nc) as tc:
      with tc.tile_pool(name="dram", bufs=1, space="DRAM") as dpool:
        bplane = [[dpool.tile([128, TOK], dt.float16, tag=f"bp{m}{j}",
                              name=f"bp{m}{j}") for j in range(4)]
                  for m in range(NREC)]
        gplane = [dpool.tile([128, TOK], dt.float16, tag=f"gp{j}",
                             name=f"gp{j}") for j in range(4)]

        with (
            tc.tile_pool(name="wpool", bufs=1) as wp,
            tc.tile_pool(name="p1x", bufs=2) as p1x,
            tc.tile_pool(name="p1o", bufs=4) as p1o,
            tc.tile_pool(name="p1ps", bufs=3, space="PSUM") as p1ps,
            tc.tile_pool(name="p2in", bufs=3) as p2in,
            tc.tile_pool(name="p2st", bufs=3) as p2st,
            tc.tile_pool(name="p2c", bufs=4) as p2c,
            tc.tile_pool(name="p2ps", bufs=2, space="PSUM") as p2ps,
        ):
            # ---- persistent weights ----
            pw = wp.tile([128, NMAT * 2 * 4 * 128], dt.float16)
            nc.sync.dma_start(pw[:], pw_d[:])
            bias = wp.tile([128, 4 * NMAT], dt.float32)
            nc.sync.dma_start(bias[:], bias_d[:])
            aw = wp.tile([128, NREC * 16 * 128], dt.float8e4)
            nc.sync.dma_start(aw[:], aw_d[:])
            iden = wp.tile([128, 128], dt.float16)
            nc.sync.dma_start(iden[:], iden_d[:])
            s0t = wp.tile([128, 64], dt.float16)
            nc.sync.dma_start(s0t[:], s0_d[:])

            # ---- phase-1 state ----
            xt_cur = [None]
            ps_cur = {}

            def p1_mm(tt, mat, j, i):
                """One (LDW+MM) pair: i-chunk i of unit (mat, j), tile tt."""
                if (mat, j, i) == (0, 0, 0):
                    xt = p1x.tile([128, 2 * TT], dt.float16, tag="xt")
                    for ic in range(2):
                        nc.sync.dma_start(xt[:, ic * TT:(ic + 1) * TT],
                                          xT_d[ic, :, tt * TT:(tt + 1) * TT])
                    xt_cur[0] = xt
                xt = xt_cur[0]
                if i == 0:
                    ps = p1ps.tile([128, TT], dt.float32, tag="p1ps",
                                   name=f"p1ps_{tt}_{mat}_{j}")
                    ps_cur[(mat, j)] = ps
                else:
                    ps = ps_cur[(mat, j)]
                blk = ((mat * 2 + i) * 4 + j) * 128
                with tc.high_priority(offset=-400):
                    nc.tensor.matmul(ps[:], pw[:, blk:blk + 128],
                                     xt[:, i * TT:(i + 1) * TT],
                                     start=(i == 0), stop=(i == 1))

            ob_cur = {}

            def p1_drain(tt, mat, j, qtr):
                """Quarter-tile psum drain on ACT (+ plane write on the
                last quarter). Quarter-sized ops bound the head-of-line
                delay they can inflict on the critical tanh ops sharing
                the in-order ACT queue (~280ns vs ~690ns for a full-tile
                drain); separate paced entries scatter them across steps."""
                ps = ps_cur[(mat, j)]
                if qtr == 0:
                    ob = p1o.tile([128, TT], dt.float16, tag="ob",
                                  name=f"ob_{tt}_{mat}_{j}")
                    ob_cur[(mat, j)] = ob
                else:
                    ob = ob_cur[(mat, j)]
                QW = TT // 4
                sl = slice(qtr * QW, (qtr + 1) * QW)
                with tc.high_priority(offset=-400):
                    if mat == NMAT - 1:  # gate
                        nc.scalar.activation(
                            ob[:, sl], ps[:, sl], AF.Sigmoid,
                            bias=bias[:, mat * 4 + j:mat * 4 + j + 1])
                    else:
                        nc.scalar.activation(
                            ob[:, sl], ps[:, sl], AF.Identity,
                            bias=bias[:, mat * 4 + j:mat * 4 + j + 1])
                if qtr == 3:
                    ps_cur.pop((mat, j))
                    plane = (gplane[j] if mat == NMAT - 1 else bplane[mat][j])
                    nc.sync.dma_start(
                        plane[:, tt * TT:(tt + 1) * TT], ob[:])

            p1_units = [(mat, j) for mat in range(NMAT) for j in range(4)]

            def p1_tile_entries(tt):
                ents = []
                for (mat, j) in p1_units:
                    ents.append(("mm", tt, mat, j, 0))
                    ents.append(("mm", tt, mat, j, 1))
                    for qtr in range(4):
                        ents.append(("dr", tt, mat, j, qtr))
                return ents

            def p1_emit(ent):
                kind, tt, mat, j, k = ent
                if kind == "mm":
                    p1_mm(tt, mat, j, k)
                else:
                    p1_drain(tt, mat, j, k)

            # prologue: tiles 0,1 fully (cover groups 0..3)
            for tt in range(min(2, NTT)):
                for ent in p1_tile_entries(tt):
                    p1_emit(ent)

            # remaining tiles paced into the step loop
            p1q = [ent for tt in range(2, NTT) for ent in p1_tile_entries(tt)]
            EPT = 6 * len(p1_units)  # entries per tile
            p1pos = [0]

            def p1_pump(upto):
                while p1pos[0] < min(upto, len(p1q)):
                    p1_emit(p1q[p1pos[0]])
                    p1pos[0] += 1

            def p1_target(g):
                # tile X (covers groups 2X, 2X+1; loaded at group 2X-1)
                # must be done by end of group 2X-2
                return EPT * min(max(0, NTT - 2), max(0, (g + 2) // 2 - 1))

            # ---- phase 2 ----
            prev = s0t
            prev_off = 0
            stw = None

            CH = W // 2  # psum-bank / half width

            def emit_group_loads(g):
                """Group DMA loads + coef tile allocs. pjg first: the prep
                quarters consume it soonest."""
                pjg = p2in.tile([128, 1024], dt.float16, tag="pjg",
                                name=f"pjg{g}")
                for j in range(4):
                    nc.sync.dma_start(pjg[:, j * 256:(j + 1) * 256],
                                      gplane[j][:, g * 256:(g + 1) * 256])
                pjb = p2in.tile([128, NREC * 4 * 256], dt.float16, tag="pjb",
                                name=f"pjb{g}")
                for m in range(NREC):
                    for j in range(4):
                        nc.sync.dma_start(
                            pjb[:, (m * 4 + j) * 256:(m * 4 + j + 1) * 256],
                            bplane[m][j][:, g * 256:(g + 1) * 256])
                # gate coef planes stored (t, j, b) so per-step slices are
                # flat [128, 64] / [128, 32] (strided DVE ops cost ~2x)
                gco = p2in.tile([128, 1024], dt.float16, tag="gco",
                                name=f"gco{g}")
                g1m = p2in.tile([128, 1024], dt.float16, tag="g1m",
                                name=f"g1m{g}")
                return (pjb, pjg, gco, g1m)

            def emit_prep_quarter(st, idx):
                """One [128,256] gate-coef prep (idx 0-3: gco quarters,
                4-7: g1m). Quarter-sized and scattered across steps so a
                prep never occupies the in-order DVE queue ahead of the
                blend chain for more than ~300ns (same head-of-line bound
                as the quarter-sized ACT drains)."""
                pjb, pjg, gco, g1m = st
                q = idx % 4
                # dst traversal is contiguous memory order (t,j,b) so the
                # write range is tracked exactly; the permuted strides live
                # on the src side only
                src = pjg[:].rearrange("p (j t b) -> p t j b",
                                       j=4, t=16)[:, 4 * q:4 * q + 4, :, :]
                with tc.high_priority(offset=-400):
                    if idx < 4:
                        dstv = gco[:, q * 256:(q + 1) * 256].rearrange(
                            "p (t j b) -> p t j b", t=4, j=4)
                        nc.vector.tensor_scalar_mul(dstv, src, gbase)
                    else:
                        dstv = g1m[:, q * 256:(q + 1) * 256].rearrange(
                            "p (t j b) -> p t j b", t=4, j=4)
                        nc.vector.tensor_scalar(dstv, src, -1.0, 1.0,
                                                ALU.mult, ALU.add)

            def finish_group(st):
                pjb, pjg, gco, g1m = st
                return (pjb[:].rearrange("p (m j t b) -> p m j t b",
                                         m=NREC, j=4, t=16, b=16),
                        gco, g1m)

            g0st = emit_group_loads(0)
            for i in range(8):
                emit_prep_quarter(g0st, i)
            ginputs = {0: finish_group(g0st)}
            gpend = {}

            for g in range(NG):
                pjbr, gcor, g1mr = ginputs.pop(g)

                p1_start = min(p1_target(g - 1), len(p1q))
                p1_end = min(p1_target(g), len(p1q))

                for tt in range(16):
                    if g + 1 < NG:
                        if tt == 8:
                            gpend[g + 1] = emit_group_loads(g + 1)
                            ginputs[g + 1] = finish_group(gpend[g + 1])
                        if tt >= 8:
                            emit_prep_quarter(gpend[g + 1], tt - 8)
                    t = g * 16 + tt
                    q = tt % 8
                    if q == 0:
                        stw = p2st.tile([128, 512], dt.float16, tag="stw")
                    if prev is s0t:
                        prevc = prev[:]
                    else:
                        prevc = prev[:, prev_off * 64:(prev_off + 1) * 64]

                    # m2 = (1-g_t) * s on Pool, early (off the DVE chain)
                    m2 = p2c.tile([128, 64], dt.float16, tag="m2")
                    nc.gpsimd.tensor_tensor(
                        m2[:], prevc, g1mr[:, tt * 64:(tt + 1) * 64],
                        ALU.mult)

                    # one psum bank per tanh half: cols (m, j%2, b)
                    psh = [p2ps.tile([128, CH], dt.float32, tag=f"ps{h}",
                                     name=f"ps{h}_{t}")
                           for h in range(2)]

                    def a_mm(j, k, last):
                        for m in range(NREC):
                            blk = ((m * 4 + j) * 4 + k) * 128
                            nc.tensor.matmul(
                                psh[j // 2][:, (m * 2 + j % 2) * 16:
                                            (m * 2 + j % 2 + 1) * 16],
                                aw[:, blk:blk + 128],
                                prevc[:, k * 16:(k + 1) * 16],
                                start=False, stop=last)

                    # bx injection (E bank first), then k-rounds, whole E
                    # bank before the L bank so E-psum completes 428ns
                    # before burst end: tanh_E then clears ACT before
                    # L-psum lands, and the E blend chain clears DVE before
                    # the L chain needs it. Within each bank: stale-chunk
                    # consumers (k2,k3) first, fresh (k0,k1) last. p1
                    # filler pumps into the two stall windows: after
                    # injects (waiting prev E chunks) and between the E
                    # bank's stale and fresh rounds (waiting prev L chunks).
                    sbudget = p1_start + (tt * (p1_end - p1_start)) // 16
                    ebudget = p1_start + ((tt + 1) * (p1_end - p1_start)) // 16
                    for h in (1, 0):
                        nc.tensor.matmul(
                            psh[h][:].rearrange("p (m j b) -> p m j b",
                                                m=NREC, j=2),
                            iden[:], pjbr[:, :, 2 * h:2 * h + 2, tt, :],
                            start=True, stop=False)
                    p1_pump(min(sbudget + 1, ebudget))
                    for h in (1, 0):
                        for k in (2, 3):
                            for j in (2 * h, 2 * h + 1):
                                a_mm(j, k, last=False)
                        if h == 1:
                            p1_pump(ebudget)
                        for ki, k in enumerate((0, 1)):
                            for j in (2 * h, 2 * h + 1):
                                a_mm(j, k, last=(ki == 1))

                    # per half (E first): tanh -> fold -> gate -> state chunk.
                    # DVE emission is interleaved across the halves
                    # (v_E, u_E, v_L, add_E, u_L, add_L) so the L chain's ops
                    # start as soon as tanh_L lands instead of queueing
                    # behind the whole E chain: E-state is needed early (next
                    # stale rounds) but has slack; L-state (fresh rounds) is
                    # the binding path.
                    fth, uh, vh = {}, {}, {}
                    for h in (1, 0):
                        ft = p2c.tile([128, NREC * 32], dt.float16, tag=f"ft{h}")
                        nc.scalar.activation(ft[:], psh[h][:], AF.Tanh,
                                             scale=1.0 / SC)
                        fth[h] = ft
                        u2 = p2c.tile([128, 32], dt.float16, tag=f"u{h}",
                                      name=f"u{h}_{t}")
                        uh[h] = u2

                    def emit_v(h):
                        ft = fth[h]
                        if NREC == 2 and rfold is not None:
                            v2 = p2c.tile([128, 32], dt.float16, tag=f"v{h}")
                            if rfold == 1.0:
                                nc.vector.tensor_tensor(v2[:], ft[:, 32:64],
                                                        ft[:, 0:32], ALU.add)
                            else:
                                nc.vector.scalar_tensor_tensor(
                                    v2[:], ft[:, 32:64], rfold, ft[:, 0:32],
                                    ALU.mult, ALU.add)
                            vh[h] = v2[:]
                        elif NREC == 2:
                            vh[h] = ft[:, 32:64]
                        else:
                            vh[h] = ft[:]

                    def emit_u(h):
                        gslc = gcor[:, tt * 64 + h * 32:tt * 64 + (h + 1) * 32]
                        nc.vector.tensor_tensor(uh[h][:], vh[h], gslc,
                                                ALU.mult)

                    def emit_add(h):
                        nc.vector.tensor_tensor(
                            stw[:, q * 64 + h * 32:q * 64 + (h + 1) * 32],
                            uh[h][:], m2[:, h * 32:(h + 1) * 32], ALU.add)

                    # E chain first, complete: with the E bank finishing
                    # early in the burst, its whole chain clears DVE before
                    # tanh_L output arrives
                    emit_v(1)
                    emit_u(1)
                    emit_add(1)
                    emit_v(0)
                    emit_u(0)
                    emit_add(0)

                    prev, prev_off = stw, q

                    # batched output DMA every 8 steps
                    if q == 7:
                        nc.sync.dma_start(
                            stg_d[:, (t - 7) * 64:(t + 1) * 64], stw[:])

                p1_pump(p1_end)
            p1_pump(len(p1q))

    nc.compile()
    return nc


def _pack_lhsT_blocks(Wm, kdim, mdim, dtype):
    """Wm: [mdim*128, kdim*128]; returns [128, kdim*mdim*128] with block
    (k, j) at cols (k*mdim+j)*128 equal to Wm[j-chunk, k-chunk].T."""
    out = np.zeros((128, kdim * mdim * 128), dtype=np.float32)
    for k in range(kdim):
        for j in range(mdim):
            blk = Wm[j * 128:(j + 1) * 128, k * 128:(k + 1) * 128].T
            out[:, (k * mdim + j) * 128:(k * mdim + j + 1) * 128] = blk
    return np.ascontiguousarray(out.astype(dtype))


def kernel(x_seq, s0, A0_w, B0_w, B0_b, A1_w, B1_w, B1_b, gate_w, gate_b,
           alpha, z, _T=None, _trace=False):
    from concourse.bass_utils import run_bass_kernel_spmd

    T = int(_T or T_FULL)
    alpha_f = float(np.asarray(alpha))
    z_i = int(np.asarray(z))
    SC = 512.0

    key = (alpha_f, z_i, T)
    if key not in _cache:
        _cache[key] = _build(alpha_f, z_i, T)
    nc = _cache[key]

    NREC = 2 if z_i != 0 else 1
    NMAT = NREC + 1

    x_seq = np.asarray(x_seq, dtype=np.float32)
    s0 = np.asarray(s0, dtype=np.float32)

    # ---- replicated weights ----
    if z_i != 0:
        bmats = [np.asarray(B0_w), np.asarray(B1_w)]
        bvecs = [np.asarray(B0_b), np.asarray(B1_b)]
        recs = [np.asarray(A0_w), np.asarray(A1_w)]
    else:
        bmats = [np.asarray(B0_w)]
        bvecs = [np.asarray(B0_b)]
        recs = [np.asarray(A0_w)]

    # phase-1 lhsT blocks: bx mats prescaled by SC, gate unscaled
    pw_parts = [_pack_lhsT_blocks(Wm.astype(np.float32) * SC, 2, 4, np.float16)
                for Wm in bmats]
    pw_parts.append(_pack_lhsT_blocks(np.asarray(gate_w).astype(np.float32),
                                      2, 4, np.float16))
    pw = np.ascontiguousarray(np.concatenate(pw_parts, axis=1))

    bias = np.zeros((128, 4 * NMAT), np.float32)
    for mi, bvec in enumerate(bvecs):
        bias[:, mi * 4:(mi + 1) * 4] = (
            bvec.astype(np.float32).reshape(4, 128).T * SC)
    bias[:, NREC * 4:(NREC + 1) * 4] = (
        np.asarray(gate_b).astype(np.float32).reshape(4, 128).T)

    aw = np.concatenate(
        [_pack_lhsT_blocks(A.astype(np.float32) * SC, 4, 4, np.float32)
         for A in recs], axis=1)
    # (m,j,k) block order: _pack gives (k*4+j); need ((m*4+j)*4+k)*128
    aw = aw.reshape(128, NREC, 4, 4, 128).transpose(0, 1, 3, 2, 4)
    aw = np.ascontiguousarray(aw.reshape(128, -1).astype(ml_dtypes.float8_e4m3))

    IDEN = np.ascontiguousarray(np.eye(128, dtype=np.float16))

    # ---- per-core inputs ----
    in_maps = []
    for c in range(N_CORES):
        bc = c * B_LOC
        xc = x_seq[bc:bc + B_LOC, :T]                       # [16, T, 256]
        xT = np.ascontiguousarray(
            xc.transpose(2, 1, 0).reshape(2, 128, T * B_LOC).astype(np.float16))
        s0c = s0[bc:bc + B_LOC]                             # [16, 512]
        s0T = np.ascontiguousarray(
            s0c.T.reshape(4, 128, B_LOC).transpose(1, 0, 2)
            .reshape(128, 64).astype(np.float16))
        in_maps.append({
            "xT": xT, "pw": pw, "bias": bias, "aw": aw, "s0T": s0T,
            "iden": IDEN,
        })

    res = run_bass_kernel_spmd(nc, in_maps, list(range(N_CORES)), trace=_trace)
    if _trace:
        kernel._last_res = res

    out = np.empty((B_FULL, T + 1, S_DIM), np.float32)
    for c in range(N_CORES):
        bc = c * B_LOC
        stg = np.asarray(res.results[c]["stg"])             # [128, T*64] f16
        out[bc:bc + B_LOC, 0] = s0[bc:bc + B_LOC]
        # stg[p, t*64 + j*16 + b] = s_{t+1}[b, j*128 + p]
        st = stg.reshape(128, T, 4, B_LOC).astype(np.float32)
        out[bc:bc + B_LOC, 1:] = (
            st.transpose(3, 1, 2, 0).reshape(B_LOC, T, S_DIM))
    return out
